# revision 1
# baseline (speedup 1.0000x reference)
"""Self-contained Trainium2 Bass kernel for nn_DenseFlashAttention_16123307229343
(GNN segment-softmax message passing). kernel(**inputs) -> np.ndarray.

Strategy: receivers sharded over 8 NeuronCores; host performs edge sharding /
layout (sort by receiver, degree-balanced 128-slot chunks, x[snd] laid out
per 128-edge tile, pre-transposed); device does all math: per-tile projection
matmuls, exp attention weights, one-hot scatter matmuls into PSUM per
receiver chunk, softmax normalization, output head-mean + w_out + residual.
"""
import sys
sys.path.insert(0, '/opt/trn_rl_repo')

"""(inlined tilefix) Patch TileContext._drain_and_barrier: the walrus in this container rejects
Drain instructions carrying >1 sync wait. Split the end-of-context drain into
one drain per wait."""
import concourse.mybir as mybir
from concourse.tile import TileContext
from concourse.vector_clock import ScopedClock

MAXW = 1

def _patched_drain_and_barrier(self, tick_clock, wait_clock):
    nc = self.nc
    drain_inst = nc.sync.drain()
    wait_clock.add_sem_waits(drain_inst.ins, ScopedClock({None: tick_clock.global_clock}))
    si = drain_inst.ins.sync_info
    waits = list(si.on_wait) if si is not None else []
    if len(waits) > MAXW:
        si.on_wait = waits[:MAXW]
        rest = waits[MAXW:]
        for i in range(0, len(rest), MAXW):
            d2 = nc.sync.drain()
            d2.ins.sync_info = mybir.SyncInfo(on_wait=rest[i:i+MAXW], on_update=[])
    nc.all_engine_barrier()
    popped = nc._tile_sem_poison_stack.pop()
    assert popped is self._sem_poison
    nc.clear_and_free_semaphores(list(self.sems.allocated().values()))
    nc.all_engine_barrier()

def install():
    TileContext._drain_and_barrier = _patched_drain_and_barrier


_ctr = [0]


def split_sync_waits(nc, maxw=1):
    """The walrus build in this container rejects instructions carrying more
    than one sync wait. Hoist extra waits onto InstNoOp carriers inserted
    immediately before the instruction on the same engine."""
    for f in nc.m.functions:
        for blk in f.blocks:
            lst = blk.instructions
            i = 0
            while i < len(lst):
                ins = lst[i]
                si = ins.sync_info
                if si is None:
                    i += 1
                    continue
                waits = list(si.on_wait)
                if len(waits) <= maxw:
                    i += 1
                    continue
                si.on_wait = waits[-maxw:]
                rest = waits[:-maxw]
                carriers = []
                for j in range(0, len(rest), maxw):
                    _ctr[0] += 1
                    nop = mybir.InstEventSemaphore(name=f"waitnop_{_ctr[0]}", ins=[], outs=[])
                    nop.engine = ins.engine
                    nop.sync_info = mybir.SyncInfo(on_wait=rest[j:j + maxw],
                                                   on_update=[])
                    nc.register_instruction(nop, overwrite=True)
                    carriers.append(nop)
                for k, nop in enumerate(carriers):
                    lst.insert(i + k, nop)
                i += len(carriers) + 1


def install_walrus_dge():
    """compile_bir_kernel's walrus invocation omits --dge-levels, leaving
    dynamic-offset DMAs mislowered. Append the enablement flag."""
    import concourse.bass_utils as bu
    if getattr(bu, '_dge_patched', False):
        return
    orig = bu.get_walrus_args
    def patched(arch, tmpdir, *, dve_root=None):
        args = orig(arch, tmpdir, dve_root=dve_root)
        return list(args) + ["--dge-levels=vector_dynamic_offsets"]
    bu.get_walrus_args = patched
    bu._dge_patched = True


"""GNN segment-softmax message passing on trn2 — host-gathered variant.

Host ships x[snd[e]] pre-transposed per 128-edge tile (input layout only).
Device: per-tile proj matmul (fp32r), exp weights, one-hot scatter matmuls
into per-chunk PSUM, normalization + output assembly. Receivers sharded
across 8 cores; no collectives, no device gather.
"""
import sys, math
import numpy as np
import ml_dtypes


import concourse.bass as bass
import concourse.bacc as bacc
import concourse.mybir as mybir
from concourse.tile import TileContext
from concourse.masks import make_identity


bf16 = ml_dtypes.bfloat16
FP32 = mybir.dt.float32
F32R = mybir.dt.float32r
BF16 = mybir.dt.bfloat16
ALU = mybir.AluOpType
ACTF = mybir.ActivationFunctionType


class Params:
    def __init__(self, N, E, F=64, H=4, NC=8, G=4):
        self.N, self.E, self.F, self.H, self.NC = N, E, F, H, NC
        self.HF = H * F
        self.RPC = N // NC
        ch = math.ceil(self.RPC / 128)
        self.CH = math.ceil(ch / G) * G
        self.SLOTS = self.CH * 128
        self.G = G
        self.NG = self.CH // G


def host_prep(p: Params, x, edge_index, edge_len):
    N, NC = p.N, p.NC
    snd = edge_index[0].astype(np.int64)
    rcv = edge_index[1].astype(np.int64)
    deg = np.bincount(rcv, minlength=N)
    order = np.argsort(rcv, kind='stable')
    starts = np.zeros(N + 1, np.int64)
    np.cumsum(deg, out=starts[1:])

    cores = []
    T_need = 0
    for k in range(NC):
        lo, hi = k * p.RPC, (k + 1) * p.RPC
        d = deg[lo:hi]
        srt = np.argsort(-d, kind='stable')
        bin_of = np.empty(p.RPC, np.int64)
        slot_of = np.empty(p.RPC, np.int64)
        ar = np.arange(p.RPC)
        bin_of[srt] = ar % p.CH
        slot_of[srt] = ar // p.CH
        bin_edge_counts = np.zeros(p.CH, np.int64)
        np.add.at(bin_edge_counts, bin_of, d)
        T_need = max(T_need, int(np.ceil(bin_edge_counts.max() / 128)))
        cores.append(dict(bin_of=bin_of, slot_of=slot_of))
    T = max(T_need, 1)

    per_core = []
    for k in range(NC):
        c = cores[k]
        lo = k * p.RPC
        TT = p.CH * T
        esnd = np.full((TT * 128,), -1, np.int64)       # -1 = dummy
        lens = np.zeros((TT * 128,), np.float32)
        roff = np.full((TT * 128,), 255.0, np.float32)  # 255 = no slot (dummy)
        fill = np.zeros(p.CH, np.int64)
        for r_local in np.argsort(c['bin_of'], kind='stable'):
            b = c['bin_of'][r_local]
            s = c['slot_of'][r_local]
            n = lo + r_local
            e0, e1 = starts[n], starts[n + 1]
            cnt = e1 - e0
            if cnt == 0:
                continue
            base = b * (T * 128) + fill[b]
            eidx = order[e0:e1]
            esnd[base:base + cnt] = snd[eidx]
            lens[base:base + cnt] = edge_len[eidx]
            roff[base:base + cnt] = s
            fill[b] += cnt
        assert fill.max() <= T * 128
        # xeT [F, TT*128]: column e = x[snd[e]] (zero for dummies)
        xe = np.zeros((TT * 128, p.F), np.float32)
        real = esnd >= 0
        xe[real] = x[esnd[real]]
        xeT = np.ascontiguousarray(xe.T)
        lenT = lens.reshape(TT, 128).T.copy()
        roffT = roff.reshape(TT, 128).T.copy().astype(bf16)
        x_rcv = np.zeros((p.SLOTS, p.F), np.float32)
        slot_global = c['bin_of'] * 128 + c['slot_of']
        x_rcv[slot_global] = x[lo:lo + p.RPC]
        x_rcvT = np.ascontiguousarray(x_rcv.T)
        per_core.append(dict(xeT=xeT, lenT=lenT, roffT=roffT,
                             x_rcv=x_rcv, x_rcvT=x_rcvT,
                             slot_global=slot_global))
    return dict(T=T, per_core=per_core)


def build_program(p: Params, T: int, use_bacc=False):
    cls = bacc.Bacc if use_bacc else bass.Bass
    nc = cls("TRN2", target_bir_lowering=False, debug=False, num_devices=p.NC)
    F, H, HF, CH, G, NG = p.F, p.H, p.HF, p.CH, p.G, p.NG
    RW = HF + H                 # 260
    S = p.SLOTS
    TT = CH * T

    x_rcv = nc.dram_tensor("x_rcv", [S, F], FP32, kind="ExternalInput").ap()
    x_rcvT = nc.dram_tensor("x_rcvT", [F, S], FP32, kind="ExternalInput").ap()
    xeT = nc.dram_tensor("xeT", [F, TT * 128], F32R, kind="ExternalInput").ap()
    wcat32 = nc.dram_tensor("wcat32", [F, HF], FP32, kind="ExternalInput").ap()
    wcatT32 = nc.dram_tensor("wcatT32", [F, HF], FP32, kind="ExternalInput").ap()
    scores32 = nc.dram_tensor("scores32", [F, 2 * H], FP32, kind="ExternalInput").ap()
    wout32 = nc.dram_tensor("wout32", [F, F], FP32, kind="ExternalInput").ap()
    rds_b = nc.dram_tensor("rds_b", [128, 1], FP32, kind="ExternalInput").ap()
    iota_in = nc.dram_tensor("iota_in", [128, 128], BF16, kind="ExternalInput").ap()
    lenT = nc.dram_tensor("lenT", [128, TT], FP32, kind="ExternalInput").ap()
    roffT = nc.dram_tensor("roffT", [128, TT], BF16, kind="ExternalInput").ap()
    y_perm = nc.dram_tensor("y_perm", [S, F], FP32, kind="ExternalOutput").ap()

    with TileContext(nc) as tc:
        import contextlib
        ctx = contextlib.ExitStack()
        with ctx:
            const = ctx.enter_context(tc.tile_pool(name="const", bufs=1))
            iota_s = const.tile([128, 128], BF16)
            nc.sync.dma_start(out=iota_s[:], in_=iota_in[:])
            wout_s = const.tile([F, F], FP32)
            nc.sync.dma_start(out=wout_s[:], in_=wout32[:])
            xrcvT_s = const.tile([F, S], FP32)
            nc.sync.dma_start(out=xrcvT_s[:], in_=x_rcvT[:])
            wcat_s32 = const.tile([F, HF], FP32)
            nc.sync.dma_start(out=wcat_s32[:], in_=wcat32[:])
            wcatT_s32 = const.tile([F, HF], FP32)
            nc.sync.dma_start(out=wcatT_s32[:], in_=wcatT32[:])
            scores_s = const.tile([F, 2 * H], FP32)
            nc.sync.dma_start(out=scores_s[:], in_=scores32[:])
            rds_s = const.tile([128, 1], FP32)
            nc.sync.dma_start(out=rds_s[:], in_=rds_b[:])
            nrds_s = const.tile([128, 1], FP32)
            nc.vector.tensor_scalar(out=nrds_s[:], in0=rds_s[:], scalar1=-1.0,
                                    scalar2=None, op0=ALU.mult)
            ident = const.tile([128, 128], FP32)
            make_identity(nc, ident[:])

            # WV = [Wcat | Vt | Vr] fp32 [F, 264]; V via 8 small matmuls
            wv_s = const.tile([F, HF + 2 * H], F32R)
            nc.vector.tensor_copy(out=wv_s[:, 0:HF], in_=wcat_s32[:])
            with tc.tile_pool(name="vps", bufs=1, space="PSUM") as vpsp:
                vps = vpsp.tile([F, 2 * H], FP32, space="PSUM")
                for j in range(2 * H):
                    h = j % H
                    nc.tensor.matmul(out=vps[:, j:j + 1],
                                     lhsT=wcatT_s32[:, h * F:(h + 1) * F],
                                     rhs=scores_s[:, j:j + 1],
                                     start=True, stop=True)
                nc.vector.tensor_copy(out=wv_s[:, HF:], in_=vps[:])
            # Wm2 = -0.5*sum_h W_h
            wm2 = const.tile([F, F], FP32)
            nc.vector.tensor_tensor(out=wm2[:], in0=wcat_s32[:, 0:F],
                                    in1=wcat_s32[:, F:2 * F], op=ALU.add)
            nc.vector.tensor_tensor(out=wm2[:], in0=wm2[:],
                                    in1=wcat_s32[:, 2 * F:3 * F], op=ALU.add)
            nc.vector.tensor_tensor(out=wm2[:], in0=wm2[:],
                                    in1=wcat_s32[:, 3 * F:4 * F], op=ALU.add)
            nc.vector.tensor_scalar(out=wm2[:], in0=wm2[:], scalar1=-0.5,
                                    scalar2=None, op0=ALU.mult)

            with tc.tile_pool(name="edge", bufs=2) as ep, \
                 tc.tile_pool(name="big", bufs=2) as bigp, \
                 tc.tile_pool(name="ret", bufs=2) as rp, \
                 tc.tile_pool(name="peps", bufs=2, space="PSUM") as peps, \
                 tc.tile_pool(name="cps", bufs=2, space="PSUM") as cps, \
                 tc.tile_pool(name="rps", bufs=1, space="PSUM") as rps:
                BT = G * T
                for g in range(NG):
                    t0 = g * BT
                    xe_s = ep.tile([F, BT * 128], F32R, tag="xe")
                    nc.sync.dma_start(out=xe_s[:],
                                      in_=xeT[:, t0 * 128:(t0 + BT) * 128])
                    len_s = ep.tile([128, BT], FP32, tag="len")
                    nc.sync.dma_start(out=len_s[:], in_=lenT[:, t0:t0 + BT])
                    ro_s = ep.tile([128, BT], BF16, tag="ro")
                    nc.sync.dma_start(out=ro_s[:], in_=roffT[:, t0:t0 + BT])
                    nlen = ep.tile([128, BT], FP32, tag="nlen")
                    nc.vector.tensor_tensor(
                        out=nlen[:], in0=len_s[:],
                        in1=nrds_s[:].to_broadcast([128, BT]), op=ALU.mult)
                    # one-hots for the whole group
                    obuf = ep.tile([128, BT, 128], BF16, tag="obuf")
                    nc.vector.tensor_tensor(
                        out=obuf[:],
                        in0=iota_s[:, None, :].to_broadcast([128, BT, 128]),
                        in1=ro_s[:, :, None].to_broadcast([128, BT, 128]),
                        op=ALU.is_equal)
                    projb = bigp.tile([128, BT, HF], BF16, tag="projb")
                    sbatch = ep.tile([128, BT, 2 * H], FP32, tag="sbatch")
                    for t in range(BT):
                        pe = peps.tile([128, HF + 2 * H], FP32, space="PSUM",
                                       tag="pe")
                        lhs = xe_s[:, t * 128:(t + 1) * 128]
                        nc.tensor.matmul(
                            out=pe[:],
                            lhsT=lhs,
                            rhs=wv_s[:],
                            start=True, stop=True)
                        # proj -> bf16 staging (alternate DVE/ACT)
                        if t % 2 == 0:
                            nc.vector.tensor_copy(out=projb[:, t, :],
                                                  in_=pe[:, 0:HF])
                        else:
                            nc.scalar.copy(out=projb[:, t, :], in_=pe[:, 0:HF])
                        nc.vector.tensor_copy(out=sbatch[:, t, :],
                                              in_=pe[:, HF:])
                    # weights: wt = exp(s_t), wr = exp(s_r - rds*len)
                    nc.vector.tensor_tensor(
                        out=sbatch[:, :, H:2 * H],
                        in0=sbatch[:, :, H:2 * H],
                        in1=nlen[:, :, None].to_broadcast([128, BT, H]),
                        op=ALU.add)
                    wts = ep.tile([128, BT, 2 * H], BF16, tag="wts")
                    nc.scalar.activation(out=wts[:], in_=sbatch[:], func=ACTF.Exp)
                    rhs1 = bigp.tile([128, BT, RW], BF16, tag="rhs1")
                    rhs2 = bigp.tile([128, BT, RW], BF16, tag="rhs2")
                    for h in range(H):
                        nc.vector.tensor_tensor(
                            out=rhs2[:, :, h * F:(h + 1) * F],
                            in0=projb[:, :, h * F:(h + 1) * F],
                            in1=wts[:, :, h:h + 1].to_broadcast([128, BT, F]),
                            op=ALU.mult)
                        nc.vector.tensor_tensor(
                            out=rhs1[:, :, h * F:(h + 1) * F],
                            in0=projb[:, :, h * F:(h + 1) * F],
                            in1=wts[:, :, H + h:H + h + 1].to_broadcast([128, BT, F]),
                            op=ALU.mult)
                    nc.vector.tensor_copy(out=rhs2[:, :, HF:RW], in_=wts[:, :, 0:H])
                    nc.vector.tensor_copy(out=rhs1[:, :, HF:RW],
                                          in_=wts[:, :, H:2 * H])
                    for cc in range(G):
                        ch = g * G + cc
                        psA = cps.tile([128, RW], FP32, space="PSUM", tag="psA")
                        psB = cps.tile([128, RW], FP32, space="PSUM", tag="psB")
                        for k in range(T):
                            t = cc * T + k
                            nc.tensor.matmul(out=psA[:], lhsT=obuf[:, t, :],
                                             rhs=rhs1[:, t, :],
                                             start=(k == 0), stop=(k == T - 1))
                            nc.tensor.matmul(out=psB[:], lhsT=obuf[:, t, :],
                                             rhs=rhs2[:, t, :],
                                             start=(k == 0), stop=(k == T - 1))
                        rden = rp.tile([128, 2 * H], FP32, tag="rden")
                        nc.vector.tensor_scalar(out=rden[:, 0:H], in0=psA[:, HF:RW],
                                                scalar1=1e-30, scalar2=None,
                                                op0=ALU.max)
                        nc.vector.tensor_scalar(out=rden[:, H:2 * H],
                                                in0=psB[:, HF:RW],
                                                scalar1=1e-30, scalar2=None,
                                                op0=ALU.max)
                        rcp = rp.tile([128, 2 * H], FP32, tag="rcp")
                        nc.vector.reciprocal(out=rcp[:], in_=rden[:])
                        nc.vector.tensor_scalar(out=rcp[:], in0=rcp[:],
                                                scalar1=0.25, scalar2=None,
                                                op0=ALU.mult)
                        mneg = rp.tile([128, 1], FP32, tag="mneg")
                        nc.vector.tensor_scalar(out=mneg[:], in0=psB[:, HF:HF + 1],
                                                scalar1=0.0, scalar2=None,
                                                op0=ALU.is_gt)
                        pm = rps.tile([128, F], FP32, space="PSUM", tag="rmisc")
                        nc.tensor.matmul(out=pm[:],
                                         lhsT=xrcvT_s[:, ch * 128:(ch + 1) * 128],
                                         rhs=wm2[:], start=True, stop=True)
                        acc = rp.tile([128, F], FP32, tag="acc")
                        nc.scalar.mul(acc[:], pm[:], mneg[:, 0:1])
                        for h in range(H):
                            nc.vector.scalar_tensor_tensor(
                                out=acc[:], in0=psA[:, h * F:(h + 1) * F],
                                scalar=rcp[:, h:h + 1], in1=acc[:],
                                op0=ALU.mult, op1=ALU.add)
                            nc.vector.scalar_tensor_tensor(
                                out=acc[:], in0=psB[:, h * F:(h + 1) * F],
                                scalar=rcp[:, H + h:H + h + 1], in1=acc[:],
                                op0=ALU.mult, op1=ALU.add)
                        psT = rps.tile([128, 128], FP32, space="PSUM", tag="rmisc")
                        nc.tensor.transpose(out=psT[0:F, :], in_=acc[:],
                                            identity=ident[:])
                        accT = rp.tile([F, 128], FP32, tag="accT")
                        nc.scalar.copy(out=accT[:], in_=psT[0:F, :])
                        psY = rps.tile([128, F], FP32, space="PSUM", tag="rmisc")
                        nc.tensor.matmul(out=psY[:], lhsT=accT[:], rhs=wout_s[:],
                                         start=True, stop=True)
                        xr_s = rp.tile([128, F], FP32, tag="xr")
                        nc.sync.dma_start(out=xr_s[:],
                                          in_=x_rcv[ch * 128:(ch + 1) * 128, :])
                        ybuf = rp.tile([128, F], FP32, tag="ybuf")
                        nc.vector.tensor_tensor(out=ybuf[:], in0=psY[:],
                                                in1=xr_s[:], op=ALU.add)
                        nc.sync.dma_start(
                            out=y_perm[ch * 128:(ch + 1) * 128, :], in_=ybuf[:])
    split_sync_waits(nc, maxw=1)
    nc.finalize()
    return nc


def make_in_maps(p: Params, meta, x, w_proj, rs, ts, rds, w_out):
    H, F = p.H, p.F
    wcat = np.ascontiguousarray(np.transpose(w_proj, (1, 0, 2)).reshape(F, H * F))
    wcatT = np.ascontiguousarray(np.transpose(w_proj, (2, 0, 1)).reshape(F, H * F))
    scores = np.concatenate([ts.T, rs.T], axis=1).astype(np.float32)
    iota = np.tile(np.arange(128, dtype=np.float32), (128, 1)).astype(bf16)
    rdsb = np.full((128, 1), np.float32(rds))
    in_maps = []
    for k in range(p.NC):
        c = meta['per_core'][k]
        in_maps.append({
            "x_rcv": c['x_rcv'], "x_rcvT": c['x_rcvT'], "xeT": c['xeT'],
            "wcat32": wcat.astype(np.float32), "wcatT32": wcatT.astype(np.float32),
            "scores32": scores, "wout32": w_out.astype(np.float32),
            "rds_b": rdsb, "iota_in": iota,
            "lenT": c['lenT'], "roffT": c['roffT'],
        })
    return in_maps


def assemble(p: Params, meta, results):
    y = np.zeros((p.N, p.F), np.float32)
    for k in range(p.NC):
        c = meta['per_core'][k]
        y[k * p.RPC:(k + 1) * p.RPC] = results[k]["y_perm"][c['slot_global']]
    return y


install()

_CACHE = {}


def kernel(x, edge_index, edge_vec, edge_len, w_proj, radial_score,
           tangential_score, radial_distance_scale, w_out):
    x = np.asarray(x, np.float32)
    edge_index = np.asarray(edge_index)
    edge_len = np.asarray(edge_len, np.float32)
    w_proj = np.asarray(w_proj, np.float32)
    rs = np.asarray(radial_score, np.float32)
    ts = np.asarray(tangential_score, np.float32)
    rds = np.float32(np.asarray(radial_distance_scale))
    w_out_ = np.asarray(w_out, np.float32)

    N, F = x.shape
    H = w_proj.shape[0]
    E = edge_index.shape[1]
    p = Params(N, E, F=F, H=H, NC=8, G=2)
    meta = host_prep(p, x, edge_index, edge_len)
    T = meta['T']
    key = (N, E, F, H, T)
    if key not in _CACHE:
        _CACHE[key] = build_program(p, T)
    nc = _CACHE[key]
    in_maps = make_in_maps(p, meta, x, w_proj, rs, ts, rds, w_out_)
    from concourse.bass_utils import run_bass_kernel_spmd
    res = run_bass_kernel_spmd(nc, in_maps, list(range(p.NC)))
    y = assemble(p, meta, [res.results[i] for i in range(p.NC)])
    return y.astype(np.float32)

